# revision 25
# baseline (speedup 1.0000x reference)
"""Trainium2 Bass kernel for nn_ActorCriticGAT (2-layer GATv2 + global-mean-pool
actor-critic head), distributed over 8 NeuronCores.

Sharding: 168 destination windows of 123 nodes are assigned to (core, slot)
pairs by sorted-size grouping (slot k gets the k-th octile of window sizes,
LPT-balanced across cores) so per-core edge counts match and chunk padding is
minimal.  Each core owns 21 windows (2583 dst rows) and all edges into them.
Per-edge work is edge-major (edges on the SBUF partition axis):

  s    = xl1[src] + xr1[dst] + edge_attr @ W1e        (PSUM, 2 matmuls)
  m    = leaky_relu(s, 0.2)                           (ScalarE Prelu)
  lg   = att . m  (per head)                          (DVE mul+reduce)
  p    = exp(lg)  (softmax without max-subtraction: logits are O(1))
  num  = sum_e p * xl1[src],  den = sum_e p           (PE one-hot scatter matmul)
  h    = relu(num / den)                              (ScalarE, scale=1/den)

The dst one-hot matrices (M: [onehot(123);ea(5)] stacked with edge_attr^T, and
D: scatter one-hot) are host-built in bf16, partition-major for contiguous DMA.
xl tables are AllGathered across cores in bf16 in 3 window-piece chunks (fired
as soon as each piece is produced, overlapping the collective with compute);
rows are fetched with SWDGE dma_gather.  Padding gather indices are spread
across the table to avoid same-row HBM hotspots.  Pool partial sums are
AllReduced; the tiny actor/critic head runs replicated on every core.
"""

import os
import sys

sys.path.insert(0, "/opt/trn_rl_repo")

import numpy as np
import ml_dtypes
from contextlib import ExitStack
from dataclasses import dataclass, field

# ---------------------------------------------------------------- problem dims
N, E, G = 20000, 320000, 64
IN, HID, HEADS, EDGE_D, ACT = 128, 256, 4, 5, 8
NEG_SLOPE = 0.2

NCORES = 8
NW = 128 - EDGE_D  # 123 dst nodes per window (+5 rows of edge_attr = K=128)
WPC = 21  # windows per core
NPC = NW * WPC  # 2583 nodes per core
NPAD = NCORES * NPC  # 20664 padded nodes
NPIECE = 3  # AllGather pieces (7 window-slots each)
WPP = WPC // NPIECE  # 7 slots per piece
RPP = WPP * NW  # 861 rows per piece per core


@dataclass
class Cfg:
    n: int = N
    e: int = E
    g: int = G
    ncores: int = NCORES
    wpc: int = WPC
    cwins: list = field(default_factory=list)  # chunks per window slot [wpc]
    debug_taps: bool = False
    stop_after: str = "full"  # p1 | ag1 | l1 | ag2 | l2 | full

    @property
    def npc(self):
        return NW * self.wpc

    @property
    def npad(self):
        return self.ncores * self.npc

    @property
    def nch(self):
        return sum(self.cwins)

    @property
    def cwmax(self):
        return max(self.cwins)


def _bf(x):
    return np.asarray(x, dtype=ml_dtypes.bfloat16)


def _f32(x):
    return np.ascontiguousarray(x, dtype=np.float32)


# ================================================================ host side
def preprocess(cfg: Cfg, x, edge_index, batch, edge_attr):
    """Build per-core input maps. Returns (in_maps, cfg with cwins filled)."""
    src = np.asarray(edge_index[0], dtype=np.int64)
    dst = np.asarray(edge_index[1], dtype=np.int64)
    ea = _f32(edge_attr)
    batch = np.asarray(batch, dtype=np.int64)

    nwin_total = cfg.ncores * cfg.wpc
    win = dst // NW  # window id of each edge
    cnt = np.bincount(win, minlength=nwin_total)
    # bucket edge ids by window
    order = np.argsort(win, kind="stable")
    bounds = np.concatenate([[0], np.cumsum(cnt)])

    # --- window -> (core, slot) assignment: sorted octile grouping + LPT
    worder = np.argsort(-cnt, kind="stable")
    wins = np.zeros((cfg.ncores, cfg.wpc), dtype=np.int64)  # window of (core, slot)
    load = np.zeros(cfg.ncores, dtype=np.int64)
    for k in range(cfg.wpc):
        grp = worder[k * cfg.ncores : (k + 1) * cfg.ncores]  # desc within group
        by_load = np.argsort(load, kind="stable")  # least-loaded first
        for i, w in enumerate(grp):
            r = by_load[i]
            wins[r, k] = w
            load[r] += cnt[w]
    cw = np.maximum(
        [max(-(-cnt[wins[r, k]] // 128) for r in range(cfg.ncores)) for k in range(cfg.wpc)],
        1,
    )
    cfg.cwins = [int(c) for c in cw]
    nch = cfg.nch
    cum = np.concatenate([[0], np.cumsum(cw)])  # chunk offset of each window slot

    # --- node -> padded-row mapping (piece-major AllGather layout)
    core_of_w = np.zeros(nwin_total, dtype=np.int64)
    slot_of_w = np.zeros(nwin_total, dtype=np.int64)
    for r in range(cfg.ncores):
        for k in range(cfg.wpc):
            core_of_w[wins[r, k]] = r
            slot_of_w[wins[r, k]] = k
    nodes_all = np.arange(N)
    w_of_n = nodes_all // NW
    off_of_n = nodes_all % NW
    # layer-1 table: rank-major linear (single AllGather)
    padrow1 = (
        core_of_w[w_of_n] * (cfg.wpc * NW) + slot_of_w[w_of_n] * NW + off_of_n
    )
    # layer-2 table: piece-major (chunked AllGather overlapping l1)
    padrow2 = (
        (slot_of_w[w_of_n] // WPP) * (cfg.ncores * RPP)
        + core_of_w[w_of_n] * RPP
        + (slot_of_w[w_of_n] % WPP) * NW
        + off_of_n
    )

    xf = _f32(x)
    in_maps = []
    for r in range(cfg.ncores):
        Mm = np.zeros((128, nch, 128), dtype=ml_dtypes.bfloat16)  # partition-major
        Dm = np.zeros((128, nch, 128), dtype=ml_dtypes.bfloat16)
        # spread padding gather indices across the table (avoid HBM hotspot)
        srcflat1 = (np.arange(nch * 128, dtype=np.int64) * 613 + r * 127) % cfg.npad
        srcflat2 = srcflat1.copy()
        for wl in range(cfg.wpc):
            w = int(wins[r, wl])
            eids = order[bounds[w] : bounds[w + 1]]
            ne = len(eids)
            ck0 = int(cum[wl])
            j = np.arange(ne)
            ck = ck0 + j // 128
            sl = j % 128
            dloc = (dst[eids] - w * NW).astype(np.int64)
            Mm[dloc, ck, sl] = 1.0
            Mm[NW:128, ck, sl] = _bf(ea[eids]).T
            Dm[sl, ck, dloc] = 1.0
            srcflat1[ck0 * 128 + j] = padrow1[src[eids]]
            srcflat2[ck0 * 128 + j] = padrow2[src[eids]]

        # gather index tiles: idx j of window w -> [j%16, off16+j//16], x8 replicate
        def mk_idx(srcflat):
            tot16 = nch * 8
            idx16 = np.zeros((16, tot16), dtype=np.int16)
            for wl in range(cfg.wpc):
                o = int(cum[wl])
                nid = int(cw[wl]) * 128
                sf = srcflat[o * 128 : o * 128 + nid]
                idx16[:, o * 8 : o * 8 + nid // 16] = sf.reshape(-1, 16).T
            return np.tile(idx16, (8, 1))

        srcidx1 = mk_idx(srcflat1)
        srcidx2 = mk_idx(srcflat2)

        # batch one-hot (mask for pooling): [128, wpc, G] partition-major
        b1 = np.zeros((128, cfg.wpc, cfg.g), dtype=ml_dtypes.bfloat16)
        # x slice, transposed, window-major
        xs = np.zeros((cfg.npc, IN), dtype=np.float32)
        for wl in range(cfg.wpc):
            w = int(wins[r, wl])
            n0 = w * NW
            nv = min(NW, max(0, N - n0))
            if nv > 0:
                xs[wl * NW : wl * NW + nv] = xf[n0 : n0 + nv]
                bb = batch[n0 : n0 + nv]
                b1[np.arange(nv), wl, bb] = 1.0
        in_maps.append(
            {
                "xT": _bf(xs.T),
                "Mmat": Mm,
                "Dmat": Dm,
                "srcidx1": srcidx1,
                "srcidx2": srcidx2,
                "b1hot": b1,
            }
        )
    return in_maps


def make_consts(W1l, W1r, W1e, att1, W2l, W2r, W2e, att2, Wp, bp, Wv, bv):
    return {
        "w1l": _bf(W1l),
        "w1r": _bf(W1r),
        "we1t": _bf(np.tile(np.asarray(W1e, np.float32)[:, None, :], (1, WPC, 1))),
        "att1bc": _bf(np.tile(np.asarray(att1).reshape(1, 1, -1), (128, 4, 1))),
        "w2l": _bf(np.asarray(W2l, np.float32).reshape(2, 128, HID).transpose(1, 0, 2)),
        "w2r": _bf(np.asarray(W2r, np.float32).reshape(2, 128, HID).transpose(1, 0, 2)),
        "we2t": _bf(np.tile(np.asarray(W2e, np.float32)[:, None, :], (1, WPC, 1))),
        "att2bc": _bf(np.tile(np.asarray(att2).reshape(1, 1, -1), (128, 4, 1))),
        "i128bf": _bf(np.eye(128)),
        "i128f": _f32(np.eye(128)),
        "wpv": _f32(
            np.concatenate([_f32(Wp), _f32(Wv)], axis=1)
            .reshape(2, 128, ACT + 1)
            .transpose(1, 0, 2)
        ),
        "bpv": _f32(
            np.tile(
                np.concatenate([_f32(bp), _f32(bv)]).reshape(1, ACT + 1), (G, 1)
            )
        ),
    }


# ================================================================ device side
def build_program(cfg: Cfg):
    import concourse.bass as bass
    import concourse.bacc as bacc
    import concourse.mybir as mybir
    import concourse.tile as tile
    from concourse import library_config

    f32, bf16, i16 = mybir.dt.float32, mybir.dt.bfloat16, mybir.dt.int16
    AF = mybir.ActivationFunctionType
    ALU = mybir.AluOpType

    wpc, nch, cwins = cfg.wpc, cfg.nch, cfg.cwins
    cwmax = cfg.cwmax
    npc, npad, g = cfg.npc, cfg.npad, cfg.g
    cum = np.concatenate([[0], np.cumsum(cwins)]).astype(int)

    nc = bacc.Bacc("TRN2", num_swdge_queues=4)

    # ---- I/O
    xT = nc.dram_tensor("xT", [IN, npc], bf16, kind="ExternalInput")
    Mmat = nc.dram_tensor("Mmat", [128, nch, 128], bf16, kind="ExternalInput")
    Dmat = nc.dram_tensor("Dmat", [128, nch, 128], bf16, kind="ExternalInput")
    srcidx1 = nc.dram_tensor("srcidx1", [128, nch * 8], i16, kind="ExternalInput")
    srcidx2 = nc.dram_tensor("srcidx2", [128, nch * 8], i16, kind="ExternalInput")
    b1hot = nc.dram_tensor("b1hot", [128, wpc, g], bf16, kind="ExternalInput")
    w1l = nc.dram_tensor("w1l", [IN, HID], bf16, kind="ExternalInput")
    w1r = nc.dram_tensor("w1r", [IN, HID], bf16, kind="ExternalInput")
    we1t = nc.dram_tensor("we1t", [EDGE_D, wpc, HID], bf16, kind="ExternalInput")
    att1bc = nc.dram_tensor("att1bc", [128, 4, HID], bf16, kind="ExternalInput")
    w2l = nc.dram_tensor("w2l", [128, 2, HID], bf16, kind="ExternalInput")
    w2r = nc.dram_tensor("w2r", [128, 2, HID], bf16, kind="ExternalInput")
    we2t = nc.dram_tensor("we2t", [EDGE_D, wpc, HID], bf16, kind="ExternalInput")
    att2bc = nc.dram_tensor("att2bc", [128, 4, HID], bf16, kind="ExternalInput")
    i128bf = nc.dram_tensor("i128bf", [128, 128], bf16, kind="ExternalInput")
    i128f = nc.dram_tensor("i128f", [128, 128], f32, kind="ExternalInput")
    wpv = nc.dram_tensor("wpv", [128, 2, ACT + 1], f32, kind="ExternalInput")
    bpv = nc.dram_tensor("bpv", [g, ACT + 1], f32, kind="ExternalInput")
    out = nc.dram_tensor("out", [g, ACT + 1], f32, kind="ExternalOutput")

    # ---- internal DRAM
    xl1_slice = nc.dram_tensor("xl1_slice", [npc, HID], bf16)
    xl1_full = nc.dram_tensor("xl1_full", [npad, HID], bf16, addr_space="Shared")
    xl2_slice = nc.dram_tensor("xl2_slice", [npc, HID], bf16)
    xl2_full = nc.dram_tensor("xl2_full", [npad, HID], bf16, addr_space="Shared")
    pp_in = nc.dram_tensor("pp_in", [g, HID + 1], f32)
    pp_out = nc.dram_tensor("pp_out", [g, HID + 1], f32, addr_space="Shared")

    taps = {}
    if cfg.debug_taps:
        taps["t_xl1full"] = nc.dram_tensor(
            "t_xl1full", [npad, HID], bf16, kind="ExternalOutput"
        )
        taps["t_h1"] = nc.dram_tensor(
            "t_h1", [wpc, 128, HID], bf16, kind="ExternalOutput"
        )
        taps["t_pp"] = nc.dram_tensor(
            "t_pp", [g, HID + 1], f32, kind="ExternalOutput"
        )

    rg = [list(range(cfg.ncores))]
    _stops = ["p1", "ag1", "l1", "ag2", "l2", "full"]

    def _on(phase):
        return _stops.index(cfg.stop_after) >= _stops.index(phase)

    with tile.TileContext(nc) as tc, ExitStack() as ctx:
        nc.gpsimd.load_library(library_config.mlp)

        consts = ctx.enter_context(tc.tile_pool(name="consts", bufs=1))
        # resident constants
        sb_att1 = consts.tile([128, 4, HID], bf16)
        nc.sync.dma_start(out=sb_att1, in_=att1bc[:, :, :])
        sb_att2 = consts.tile([128, 4, HID], bf16)
        nc.sync.dma_start(out=sb_att2, in_=att2bc[:, :, :])
        sb_ibf = consts.tile([128, 128], bf16)
        nc.sync.dma_start(out=sb_ibf, in_=i128bf[:, :])
        sb_if = consts.tile([128, 128], f32)
        nc.sync.dma_start(out=sb_if, in_=i128f[:, :])
        sb_w1l = consts.tile([IN, HID], bf16)
        nc.sync.dma_start(out=sb_w1l, in_=w1l[:, :])
        sb_w1r = consts.tile([IN, HID], bf16)
        nc.sync.dma_start(out=sb_w1r, in_=w1r[:, :])
        sb_w2l = consts.tile([128, 2, HID], bf16)
        nc.sync.dma_start(out=sb_w2l, in_=w2l[:, :, :])
        sb_w2r = consts.tile([128, 2, HID], bf16)
        nc.sync.dma_start(out=sb_w2r, in_=w2r[:, :, :])
        sb_idx1 = consts.tile([128, nch * 8], i16)
        nc.sync.dma_start(out=sb_idx1, in_=srcidx1[:, :])
        sb_idx2 = consts.tile([128, nch * 8], i16)
        nc.sync.dma_start(out=sb_idx2, in_=srcidx2[:, :])
        sb_b1h = consts.tile([128, wpc, g], bf16)
        nc.sync.dma_start(out=sb_b1h, in_=b1hot[:, :, :])
        sb_wpv = consts.tile([128, 2, ACT + 1], f32)
        nc.sync.dma_start(out=sb_wpv, in_=wpv[:, :, :])
        sb_bpv = consts.tile([g, ACT + 1], f32)
        nc.sync.dma_start(out=sb_bpv, in_=bpv[:, :])
        sb_xt = consts.tile([IN, npc], bf16)
        nc.sync.dma_start(out=sb_xt, in_=xT[:, :])
        # persistent rw tables: rows 0:NW = xr (written later), NW:128 = We
        rw1_sb = consts.tile([128, wpc, HID], bf16)
        nc.sync.dma_start(out=rw1_sb[NW:128, :, :], in_=we1t[:, :, :])
        rw2_sb = consts.tile([128, wpc, HID], bf16)
        nc.sync.dma_start(out=rw2_sb[NW:128, :, :], in_=we2t[:, :, :])

        # ---------------- P1: layer-1 projections (own slice)
        _sid, _ = nc.enter_named_scope("p1", False)
        with tc.tile_pool(name="p1", bufs=3) as p1, tc.tile_pool(
            name="p1ps", bufs=4, space="PSUM"
        ) as p1ps:
            for w in range(wpc):
                ps_l = p1ps.tile([128, HID], f32)
                ps_r = p1ps.tile([128, HID], f32)
                xtw = sb_xt[:, w * NW : (w + 1) * NW]
                nc.tensor.matmul(ps_l[0:NW, :], xtw, sb_w1l, start=True, stop=True)
                nc.tensor.matmul(ps_r[0:NW, :], xtw, sb_w1r, start=True, stop=True)
                xls = p1.tile([128, HID], bf16)
                nc.scalar.copy(xls[0:NW, :], ps_l[0:NW, :])
                nc.scalar.copy(rw1_sb[0:NW, w, :], ps_r[0:NW, :])
                nc.sync.dma_start(
                    out=xl1_slice[w * NW : (w + 1) * NW, :], in_=xls[0:NW, :]
                )
            if _on("ag1"):
                nc.gpsimd.collective_compute(
                    "AllGather",
                    mybir.AluOpType.bypass,
                    ins=[xl1_slice[:, :]],
                    outs=[xl1_full[:, :]],
                    replica_groups=rg,
                )
        nc.leave_named_scope("p1", _sid, False)
        if cfg.debug_taps and _on("ag1"):
            nc.sync.dma_start(out=taps["t_xl1full"][:, :], in_=xl1_full[:, :])

        # ---------------- edge phase (shared for both layers)
        def edge_phase(layer):
            table = xl1_full if layer == 1 else xl2_full
            sb_idx = sb_idx1 if layer == 1 else sb_idx2
            rw_sb = rw1_sb if layer == 1 else rw2_sb
            attbc = sb_att1 if layer == 1 else sb_att2
            nheads = HEADS if layer == 1 else 1
            vw = HID + nheads  # V width / nd width
            lctx = ExitStack()
            ep = lctx.enter_context(tc.tile_pool(name=f"ep{layer}", bufs=2))
            sm = lctx.enter_context(tc.tile_pool(name=f"sm{layer}", bufs=3))
            ps_s = lctx.enter_context(
                tc.tile_pool(name=f"pss{layer}", bufs=2, space="PSUM")
            )
            ps_nd = lctx.enter_context(
                tc.tile_pool(name=f"psnd{layer}", bufs=1, space="PSUM")
            )
            if layer == 1:
                ps_pr = lctx.enter_context(
                    tc.tile_pool(name="pspr", bufs=1, space="PSUM")
                )
            else:
                ps_pool = lctx.enter_context(
                    tc.tile_pool(name="pspool", bufs=1, space="PSUM")
                )
                pool_ps = ps_pool.tile([g, HID + 1], f32)

            for w in range(wpc):
                cw = cwins[w]
                ck0 = int(cum[w])
                xlg = ep.tile([128, cwmax, HID], bf16, tag="xlg")
                nsplit = min(4, cw)
                gsz = [cw // nsplit + (1 if i < cw % nsplit else 0) for i in range(nsplit)]
                go = 0
                for gi, gs in enumerate(gsz):
                    nc.gpsimd.dma_gather(
                        xlg[:, go : go + gs, :],
                        table[:, :],
                        sb_idx[:, (ck0 + go) * 8 : (ck0 + go + gs) * 8],
                        gs * 128,
                        gs * 128,
                        HID,
                        single_packet=False,
                        queue_num=gi,
                    )
                    go += gs
                msb = ep.tile([128, cwmax, 128], bf16, tag="msb")
                nc.sync.dma_start(out=msb[:, 0:cw, :], in_=Mmat[:, ck0 : ck0 + cw, :])
                dsb = ep.tile([128, cwmax, 128], bf16, tag="dsb")
                nc.sync.dma_start(out=dsb[:, 0:cw, :], in_=Dmat[:, ck0 : ck0 + cw, :])
                rw = rw_sb[:, w, :]

                nd = ps_nd.tile([128, vw], f32)
                for g0 in range(0, cw, 4):
                    gl = min(4, cw - g0)
                    m_g = sm.tile([128, 4, HID], bf16, tag="m")
                    ps = ps_s.tile([128, 4, HID], f32)
                    for ci in range(gl):
                        c = g0 + ci
                        nc.tensor.matmul(
                            ps[:, ci, :], msb[:, c, :], rw, start=True, stop=False
                        )
                        nc.tensor.matmul(
                            ps[:, ci, :],
                            sb_ibf,
                            xlg[:, c, :],
                            start=False,
                            stop=True,
                        )
                    nc.scalar.activation(
                        m_g[:, 0:gl, :],
                        ps[:, 0:gl, :],
                        AF.Prelu,
                        alpha=NEG_SLOPE,
                    )
                    # batched attention logits over the chunk group (3D APs)
                    gh = gl * nheads
                    wv_g = sm.tile([128, 4, HID], bf16, tag="wv")
                    K = HID // nheads
                    wv3 = wv_g[:, 0:gl, :].rearrange("p c (h k) -> p (c h) k", h=nheads)
                    m3 = m_g[:, 0:gl, :].rearrange("p c (h k) -> p (c h) k", h=nheads)
                    at3 = attbc[:, 0:gl, :].rearrange(
                        "p c (h k) -> p (c h) k", h=nheads
                    )
                    nc.vector.tensor_mul(wv3, m3, at3)
                    lgf = sm.tile([128, 4 * nheads], f32, tag="lg")
                    nc.vector.tensor_reduce(
                        lgf[:, 0:gh],
                        wv3,
                        axis=mybir.AxisListType.X,
                        op=ALU.add,
                    )
                    expb = sm.tile([128, 4 * nheads], bf16, tag="expb")
                    nc.scalar.activation(expb[:, 0:gh], lgf[:, 0:gh], AF.Exp)
                    # batched v build: v = exp (bcast over K) * xlg ; exp col appended
                    v4 = sm.tile([128, 4, vw], bf16, tag="v")
                    nc.vector.tensor_mul(
                        v4[:, 0:gl, 0:HID].rearrange("p c (h k) -> p c h k", h=nheads),
                        xlg[:, g0 : g0 + gl, :].rearrange(
                            "p c (h k) -> p c h k", h=nheads
                        ),
                        expb[:, 0:gh]
                        .rearrange("p (c h) -> p c h", h=nheads)
                        .unsqueeze(3)
                        .broadcast_to((128, gl, nheads, K)),
                    )
                    nc.vector.tensor_copy(
                        v4[:, 0:gl, HID:vw],
                        expb[:, 0:gh].rearrange("p (c h) -> p c h", h=nheads),
                    )
                    for ci in range(gl):
                        c = g0 + ci
                        nc.tensor.matmul(
                            nd,
                            dsb[:, c, :],
                            v4[:, ci, :],
                            start=(c == 0),
                            stop=(c == cw - 1),
                        )

                # ---- window epilogue: h = relu(num/den) via ScalarE (PSUM-side)
                dent = sm.tile([128, nheads], f32, tag="dent")
                nc.vector.tensor_scalar_add(dent, nd[:, HID:vw], 1e-16)
                rec = sm.tile([128, nheads], f32, tag="rec")
                nc.vector.reciprocal(rec, dent)
                if layer == 1:
                    hb = sm.tile([128, HID], bf16, tag="hb")
                else:
                    h2v = sm.tile([128, HID + 1], bf16, tag="h2v")
                    hb = h2v[:, 0:HID]
                C = HID // nheads
                for h in range(nheads):
                    nc.scalar.activation(
                        hb[:, h * C : (h + 1) * C],
                        nd[:, h * C : (h + 1) * C],
                        AF.Relu,
                        scale=rec[:, h : h + 1],
                    )
                if layer == 1:
                    if cfg.debug_taps:
                        nc.sync.dma_start(out=taps["t_h1"][w, :, :], in_=hb)
                    h1t = sm.tile([128, 2, 128], bf16, tag="h1t")
                    for j in range(2):
                        tp = ps_s.tile([128, 128], bf16)
                        nc.tensor.transpose(
                            tp, hb[:, j * 128 : (j + 1) * 128], sb_ibf
                        )
                        nc.scalar.copy(h1t[:, j, :], tp)
                    ps_lr = ps_pr.tile([128, 2, HID], f32)
                    ps_xl2 = ps_lr[:, 0, :]
                    ps_xr2 = ps_lr[:, 1, :]
                    for j in range(2):
                        nc.tensor.matmul(
                            ps_xl2,
                            h1t[:, j, :],
                            sb_w2l[:, j, :],
                            start=(j == 0),
                            stop=(j == 1),
                        )
                    for j in range(2):
                        nc.tensor.matmul(
                            ps_xr2,
                            h1t[:, j, :],
                            sb_w2r[:, j, :],
                            start=(j == 0),
                            stop=(j == 1),
                        )
                    xl2s = sm.tile([128, HID], bf16, tag="xl2s")
                    nc.scalar.copy(xl2s, ps_xl2)
                    nc.scalar.copy(rw2_sb[0:NW, w, :], ps_xr2[0:NW, :])
                    nc.sync.dma_start(
                        out=xl2_slice[w * NW : (w + 1) * NW, :], in_=xl2s[0:NW, :]
                    )
                    if _on("ag2") and (w + 1) % WPP == 0:
                        k = (w + 1) // WPP - 1
                        nc.gpsimd.collective_compute(
                            "AllGather",
                            mybir.AluOpType.bypass,
                            ins=[xl2_slice[k * RPP : (k + 1) * RPP, :]],
                            outs=[
                                xl2_full[
                                    k * cfg.ncores * RPP : (k + 1) * cfg.ncores * RPP,
                                    :,
                                ]
                            ],
                            replica_groups=rg,
                        )
                else:
                    nc.vector.memset(h2v[:, HID : HID + 1], 1.0)
                    if os.environ.get("KERNEL_NO_POOL", "0") != "1":
                        nc.tensor.matmul(
                            pool_ps,
                            sb_b1h[:, w, :],
                            h2v,
                            start=(w == 0),
                            stop=(w == wpc - 1),
                        )
            if layer == 2:
                pps = sm.tile([g, HID + 1], f32, tag="pps")
                nc.scalar.copy(pps, pool_ps)
                nc.sync.dma_start(out=pp_in[:, :], in_=pps)
            lctx.close()

        if _on("l1"):
            _sid, _ = nc.enter_named_scope("l1", False)
            edge_phase(1)
            nc.leave_named_scope("l1", _sid, False)

        if _on("l2"):
            _sid, _ = nc.enter_named_scope("l2", False)
            edge_phase(2)
            nc.leave_named_scope("l2", _sid, False)

        if not _on("full"):
            with tc.tile_pool(name="dummy", bufs=1) as dp:
                ob = dp.tile([g, ACT + 1], f32)
                nc.vector.memset(ob, 0.0)
                nc.sync.dma_start(out=out[:, :], in_=ob)
            nc_done = True
        else:
            nc_done = False

        # ---------------- P6: AllReduce pool partials
        if not nc_done:
            _sid, _ = nc.enter_named_scope("ar", False)
            nc.gpsimd.collective_compute(
                "AllReduce",
                mybir.AluOpType.add,
                ins=[pp_in[:, :]],
                outs=[pp_out[:, :]],
                replica_groups=rg,
            )
            nc.leave_named_scope("ar", _sid, False)
            if cfg.debug_taps:
                nc.sync.dma_start(out=taps["t_pp"][:, :], in_=pp_out[:, :])

        # ---------------- P7: head (replicated)
        if not nc_done:
          with tc.tile_pool(name="hd", bufs=1) as hd, tc.tile_pool(
            name="hdps", bufs=2, space="PSUM"
          ) as hdps:
            pp = hd.tile([g, HID + 1], f32)
            nc.sync.dma_start(out=pp, in_=pp_out[:, :])
            cnt = hd.tile([g, 1], f32)
            nc.vector.tensor_scalar_max(cnt, pp[:, HID : HID + 1], 1.0)
            rcnt = hd.tile([g, 1], f32)
            nc.vector.reciprocal(rcnt, cnt)
            gt = hd.tile([g, HID], f32)
            nc.vector.tensor_scalar_mul(gt, pp[:, 0:HID], rcnt[:, 0:1])
            gT = hd.tile([128, 2, g], f32)
            for j in range(2):
                tp = hdps.tile([128, g], f32)
                nc.tensor.transpose(
                    tp, gt[:, j * 128 : (j + 1) * 128], sb_if[0:g, 0:g]
                )
                nc.vector.tensor_copy(gT[:, j, :], tp)
            ps_o = hdps.tile([g, ACT + 1], f32)
            for j in range(2):
                nc.tensor.matmul(
                    ps_o, gT[:, j, :], sb_wpv[:, j, :], start=(j == 0), stop=(j == 1)
                )
            ob = hd.tile([g, ACT + 1], f32)
            nc.vector.tensor_add(ob, ps_o, sb_bpv)
            nc.sync.dma_start(out=out[:, :], in_=ob)

    nc.finalize()
    return nc


# ================================================================ entry point
_CACHE = {}


def kernel(
    x,
    edge_index,
    batch,
    edge_attr,
    W1l,
    W1r,
    W1e,
    att1,
    b1,
    W2l,
    W2r,
    W2e,
    att2,
    b2,
    Wp,
    bp,
    Wv,
    bv,
    _trace=False,
):
    from concourse.bass_utils import run_bass_kernel_spmd

    cfg = Cfg(stop_after=os.environ.get("STOP_AFTER", "full"))
    in_maps = preprocess(cfg, x, edge_index, batch, edge_attr)
    consts = make_consts(W1l, W1r, W1e, att1, W2l, W2r, W2e, att2, Wp, bp, Wv, bv)
    for m in in_maps:
        m.update(consts)

    key = (tuple(cfg.cwins), cfg.stop_after)
    if key not in _CACHE:
        _CACHE[key] = build_program(cfg)
    nc = _CACHE[key]

    res = run_bass_kernel_spmd(nc, in_maps, list(range(cfg.ncores)), trace=_trace)
    out = np.asarray(res.results[0]["out"], dtype=np.float32)
    if _trace:
        kernel.last_exec_time_ns = res.exec_time_ns
        kernel.last_results = res
    return out
